# revision 3
# baseline (speedup 1.0000x reference)
"""CBOW negative-sampling loss kernel for 8 TRN2 NeuronCores.

Strategy (data-parallel, per sharding hint):
  - Shard the batch (B=16384) across 8 cores -> 2048 rows/core.
  - Replicate both embedding tables in each core's DRAM.
  - Per core: 336 indirect DMAs gather the 43008 embedding rows
    (the qPoolDynamic ucode consumes one index per partition per
    instruction = 128 rows / 64KB each).  The single dynamic queue
    serializes on per-DMA completion (~1.9us/instruction), so the
    DMAs are spread round-robin over 4 SWDGE queues
    (qPoolDynamic{,1,2,3}) whose rings drain concurrently.
  - DVE computes per-row context sums and score dot-products; ACT
    runs one batched sigmoid per chunk and a single final
    Ln+row-accumulate; each core DMAs out a [128,1] partial sum and
    the host does the final scalar reduction.
"""

import numpy as np

import concourse.bacc as bacc
import concourse.bass as bass
import concourse.mybir as mybir
import concourse.tile as tile
from concourse.bass_utils import run_bass_kernel_spmd

VOCAB = 100000
DIM = 128
B = 16384
CWIN = 10
K = 10
EPS = 1e-9
NCORES = 8
P = 128
BPC = B // NCORES            # 2048 batch rows per core
NTILES = BPC // P            # 16 tiles of 128 rows
CHUNK = 2                    # batch-tiles per gather chunk
NCHUNKS = NTILES // CHUNK
NIDX = CWIN + 1 + K          # 21 lookups per batch row
NQUEUES = 4

F32 = mybir.dt.float32
MULT = mybir.AluOpType.mult
ADD = mybir.AluOpType.add
AX_X = mybir.AxisListType.X
SIGMOID = mybir.ActivationFunctionType.Sigmoid
LN = mybir.ActivationFunctionType.Ln

GATHER_BUFS = 3


def spread_dynamic_queues(nc, nq=NQUEUES):
    """Round-robin every qPoolDynamic InstDMACopy over the nq SWDGE
    queues (indirect_dma_start pins queue 0; the rings drain
    concurrently, so spreading them ~nq-x's gather throughput)."""
    k = 0
    for f in nc.m.functions:
        for blk in f.blocks:
            for i in blk.instructions:
                if isinstance(i, mybir.InstDMACopy) and i.queue.startswith(
                    "qPoolDynamic"
                ):
                    i.queue = f"qPoolDynamic{k % nq if k % nq else ''}"
                    k += 1
    return k


def build_kernel_body(tc, idx, in_emb, out_emb, usum):
    """Emit the per-core program.

    idx:    [P, NTILES*NIDX] int32 SBUF-layout indices.  Cols 0..159 are
            context lookups (free pos t*10+j -> in_emb row for batch row
            t*128+partition, window slot j); cols 160..335 are target+neg
            lookups (free pos 160 + t*11 + j -> out_emb row; j=0 target,
            j=1..10 negatives).
    usum:   [P, 1] f32; per-partition sum over this core's 16 tiles of
            log(sigmoid(pos)+eps) + sum_k log(sigmoid(-neg_k)+eps).
    """
    nc = tc.nc
    ctx_cols = NTILES * CWIN          # 160
    tn_cols_all = NTILES * (K + 1)    # 176
    with (
        tc.tile_pool(name="io", bufs=1) as io_pool,
        tc.tile_pool(name="gather", bufs=GATHER_BUFS) as gpool,
        tc.tile_pool(name="work", bufs=2) as wpool,
    ):
        idx_t = io_pool.tile([P, NTILES * NIDX], mybir.dt.int32)
        nc.sync.dma_start(out=idx_t[:], in_=idx[:, :])

        eps_t = io_pool.tile([P, 1], F32)
        nc.vector.memset(eps_t[:], EPS)

        s_all = io_pool.tile([P, tn_cols_all], F32)
        sig_all = io_pool.tile([P, tn_cols_all], F32)
        us = io_pool.tile([P, 1], F32)

        for c in range(NCHUNKS):
            ctx_g = gpool.tile([P, CHUNK * CWIN * DIM], F32, tag="ctx")
            tn_g = gpool.tile([P, CHUNK * (K + 1) * DIM], F32, tag="tn")
            c0 = c * CHUNK * CWIN
            t0 = ctx_cols + c * CHUNK * (K + 1)
            # one 128-row gather per (tile, lookup) column
            for q in range(CHUNK * CWIN):
                nc.gpsimd.indirect_dma_start(
                    out=ctx_g[:, q * DIM : (q + 1) * DIM],
                    out_offset=None,
                    in_=in_emb[:, :],
                    in_offset=bass.IndirectOffsetOnAxis(
                        ap=idx_t[:, c0 + q : c0 + q + 1], axis=0
                    ),
                )
            for q in range(CHUNK * (K + 1)):
                nc.gpsimd.indirect_dma_start(
                    out=tn_g[:, q * DIM : (q + 1) * DIM],
                    out_offset=None,
                    in_=out_emb[:, :],
                    in_offset=bass.IndirectOffsetOnAxis(
                        ap=idx_t[:, t0 + q : t0 + q + 1], axis=0
                    ),
                )

            for b in range(CHUNK):
                t_idx = c * CHUNK + b
                bc = b * CWIN * DIM        # base into ctx_g
                bt = b * (K + 1) * DIM     # base into tn_g
                sc = t_idx * (K + 1)       # base col into s_all

                # context sum over the 10 window rows (tree of adds)
                a1 = wpool.tile([P, 5 * DIM], F32, tag="a1")
                nc.vector.tensor_add(
                    a1[:], ctx_g[:, bc : bc + 5 * DIM],
                    ctx_g[:, bc + 5 * DIM : bc + 10 * DIM],
                )
                b1 = wpool.tile([P, 2 * DIM], F32, tag="b1")
                nc.vector.tensor_add(
                    b1[:], a1[:, 0 : 2 * DIM], a1[:, 2 * DIM : 4 * DIM]
                )
                csum = wpool.tile([P, DIM], F32, tag="csum")
                nc.vector.tensor_add(csum[:], b1[:, 0:DIM], b1[:, DIM : 2 * DIM])
                nc.vector.tensor_add(csum[:], csum[:], a1[:, 4 * DIM : 5 * DIM])

                # scores: s[:,0] = sum_d csum*tgt ; s[:,1+k] = sum_d csum*neg_k
                prod = wpool.tile([P, (K + 1) * DIM], F32, tag="prod")
                prod3 = prod[:].rearrange("p (k d) -> p k d", d=DIM)
                tn3 = tn_g[:, bt : bt + (K + 1) * DIM].rearrange(
                    "p (k d) -> p k d", d=DIM
                )
                csum_b = csum[:][:, None, :].to_broadcast([P, K + 1, DIM])
                nc.vector.tensor_tensor(prod3, tn3, csum_b, MULT)
                nc.vector.tensor_reduce(
                    out=s_all[:, sc : sc + 1 + K], in_=prod3, axis=AX_X, op=ADD
                )
                # flip the target column so sigmoid(-0.1*s) = sigmoid(+pos)
                nc.vector.tensor_scalar_mul(
                    s_all[:, sc : sc + 1], s_all[:, sc : sc + 1], -1.0
                )

            # batched sigmoid for this chunk; the /10 context-mean is
            # folded into the activation scale
            cs = c * CHUNK * (K + 1)
            ce = (c + 1) * CHUNK * (K + 1)
            nc.scalar.activation(
                sig_all[:, cs:ce], s_all[:, cs:ce], SIGMOID, scale=-0.1
            )

        # one Ln over all 176 cols, row-accumulated into us
        lnv = io_pool.tile([P, tn_cols_all], F32)
        nc.scalar.activation(
            lnv[:], sig_all[:], LN, bias=eps_t[:], accum_out=us[:, 0:1]
        )

        nc.sync.dma_start(out=usum[:, :], in_=us[:])


def build_nc():
    nc = bacc.Bacc(
        "TRN2",
        target_bir_lowering=False,
        debug=False,
        enable_asserts=False,
        num_devices=NCORES,
        num_swdge_queues=NQUEUES,
    )
    idx = nc.dram_tensor(
        "idx", [P, NTILES * NIDX], mybir.dt.int32, kind="ExternalInput"
    )
    in_emb = nc.dram_tensor("in_emb", [VOCAB, DIM], F32, kind="ExternalInput")
    out_emb = nc.dram_tensor("out_emb", [VOCAB, DIM], F32, kind="ExternalInput")
    usum = nc.dram_tensor("usum", [P, 1], F32, kind="ExternalOutput")
    with tile.TileContext(nc) as tc:
        build_kernel_body(tc, idx.ap(), in_emb.ap(), out_emb.ap(), usum.ap())
    spread_dynamic_queues(nc)
    nc.compile()
    return nc


def make_in_maps(context, target, negatives, in_emb, out_emb):
    context = np.asarray(context).astype(np.int32)
    target = np.asarray(target).astype(np.int32)
    negatives = np.asarray(negatives).astype(np.int32)
    in_emb = np.ascontiguousarray(np.asarray(in_emb, dtype=np.float32))
    out_emb = np.ascontiguousarray(np.asarray(out_emb, dtype=np.float32))
    tn_full = np.concatenate([target[:, None], negatives], axis=1)  # [B, 11]
    in_maps = []
    for c in range(NCORES):
        ctx_sl = context[c * BPC : (c + 1) * BPC]  # [2048, 10]
        tn_sl = tn_full[c * BPC : (c + 1) * BPC]   # [2048, 11]
        ctx_tiles = (
            ctx_sl.reshape(NTILES, P, CWIN)
            .transpose(1, 0, 2)
            .reshape(P, NTILES * CWIN)
        )
        tn_tiles = (
            tn_sl.reshape(NTILES, P, K + 1)
            .transpose(1, 0, 2)
            .reshape(P, NTILES * (K + 1))
        )
        tiles = np.concatenate([ctx_tiles, tn_tiles], axis=1)  # [P, 336]
        in_maps.append(
            {
                "idx": np.ascontiguousarray(tiles),
                "in_emb": in_emb,
                "out_emb": out_emb,
            }
        )
    return in_maps


OUT_NAMES = ["usum"]


def reduce_outputs(outs):
    return float(outs["usum"].astype(np.float64).sum())


_NC_CACHE = []
LAST_RESULT = None  # BassKernelResults of the most recent run (for profiling)


def kernel(**inputs) -> np.ndarray:
    global LAST_RESULT
    in_maps = make_in_maps(
        inputs["context"],
        inputs["target"],
        inputs["negatives"],
        inputs["in_emb"],
        inputs["out_emb"],
    )
    if not _NC_CACHE:
        _NC_CACHE.append(build_nc())
    nc = _NC_CACHE[0]
    res = run_bass_kernel_spmd(nc, in_maps, core_ids=list(range(NCORES)))
    LAST_RESULT = res
    total = sum(reduce_outputs(r) for r in res.results)
    return np.array(-total / B, dtype=np.float32)
